# revision 45
# baseline (speedup 1.0000x reference)
"""Sparse (masked) multi-head attention on 8 Trainium2 NeuronCores.

Problem: nodes [2,2048,512], edge_mask [2,2048,2048] (bool),
q/kv/o linear layers with H=8 heads of DH=64.

Sharding: batch x head-group.  Core c handles batch b = c//4 and head group
g = c%4 (heads 2g, 2g+1 = inner columns g*128:(g+1)*128).  Each core
computes its two heads' attention over the full sequence plus its partial
contribution to the output projection; the host sums the 4 partials per
batch and adds bo.

Per-core dataflow (all matmuls bf16 inputs, fp32 PSUM accumulation):
  qT/kT [dh=128, N]  = wq_sliceT @ nodesT (+bias)        (dh on partitions)
  v     [N, dh=128]  = nodesT.T @ wv_slice (+bias via ones-row matmul)
  per head h: simT[j,i] = kT_h.T @ qT_h                  (j on partitions)
              PT = exp(simT * DH**-0.5)   (ScalarE, free scale, bf16 out)
              PT *= maskT                  (VectorE, bf16 2x mode)
              numT[0:64,i] / den[64,i] = [v_h | 1].T @ PT  (ones col -> denom)
              attnT_h = numT * recip(den)  (recip + DMA partition-broadcast)
  out[i,:] += attnT.T @ wo_slice           (contraction over both heads)
"""
import numpy as np
import ml_dtypes

import concourse.bass as bass
import concourse.bacc as bacc
import concourse.tile as tile
from concourse import mybir
from concourse.bass_utils import run_bass_kernel_spmd
from bass_rust import add_dep_helper

B, N, DIM = 2, 2048, 512
H, DH = 8, 64
INNER = H * DH
SCALE = DH ** -0.5
NCORES = 8
HEADS_PER_CORE = 2
HG = 128            # inner columns per core (2 heads x 64)
NJB = N // 128      # 16 j-blocks
NISL = N // 512     # 4 i-slices of 512
NC_DIM = DIM // 128  # 4 contraction chunks over DIM

BF16 = mybir.dt.bfloat16
F32 = mybir.dt.float32
ts = bass.ts


def _build():
    nc = bacc.Bacc(monotonic_sem_count=0)
    nT_d = nc.declare_dram_parameter("nodesT", [DIM, N], BF16, isOutput=False)
    maskT_d = nc.declare_dram_parameter("maskT", [N, N], BF16, isOutput=False)
    wq_d = nc.declare_dram_parameter("wq_s", [DIM, HG], BF16, isOutput=False)
    wk_d = nc.declare_dram_parameter("wk_s", [DIM, HG], BF16, isOutput=False)
    wv_d = nc.declare_dram_parameter("wv_s", [DIM, HG], BF16, isOutput=False)
    wo_d = nc.declare_dram_parameter("wo_s", [HG, DIM], BF16, isOutput=False)
    bq_d = nc.declare_dram_parameter("bq_s", [HG, 1], F32, isOutput=False)
    bk_d = nc.declare_dram_parameter("bk_s", [HG, 1], F32, isOutput=False)
    out_d = nc.declare_dram_parameter("out", [N, DIM], BF16, isOutput=True)

    with tile.TileContext(nc) as tc:
        with (
            tc.tile_pool(name="persist", bufs=1) as persist,
            tc.tile_pool(name="ptp", bufs=8) as ptp,
            tc.tile_pool(name="denp", bufs=1) as denp,
            tc.tile_pool(name="outp", bufs=4) as outp,
            # PSUM is 8 banks total; two 4-bank pools, one slot each, shared
            # across phases: psA = {q/k proj, sim, even o-proj}, psB = {v proj,
            # num, odd o-proj}.
            tc.tile_pool(name="psA", bufs=1, space="PSUM") as psA,
            tc.tile_pool(name="psB", bufs=1, space="PSUM") as psB,
        ):
            # ---- loads ----
            # small weight/bias transfers issue first (they gate the
            # projections); nT chunks next; all on the Activation HWDGE
            # whose sequencer is idle until the first exp
            wq = persist.tile([128, NC_DIM, HG], BF16)
            nc.scalar.dma_start(
                out=wq[:], in_=wq_d.rearrange("(c p) m -> p c m", p=128)
            )
            nT = persist.tile([128, NC_DIM, N], BF16)
            nT_r = nT_d.rearrange("(c p) n -> p c n", p=128)
            nt_dmas = []
            for c in range(NC_DIM):
                d = nc.scalar.dma_start(out=nT[:, c, :], in_=nT_r[:, c, :])
                nt_dmas.append(d)
            wk = persist.tile([128, NC_DIM, HG], BF16)
            nc.scalar.dma_start(
                out=wk[:], in_=wk_d.rearrange("(c p) m -> p c m", p=128)
            )
            wv = persist.tile([128, NC_DIM, HG], BF16)
            nc.scalar.dma_start(
                out=wv[:], in_=wv_d.rearrange("(c p) m -> p c m", p=128)
            )
            wo = persist.tile([HG, DIM], BF16)
            nc.scalar.dma_start(out=wo[:], in_=wo_d[:])
            bq = persist.tile([HG, 1], F32)
            nc.scalar.dma_start(out=bq[:], in_=bq_d[:])
            bk = persist.tile([HG, 1], F32)
            nc.scalar.dma_start(out=bk[:], in_=bk_d[:])
            # mask transfers wait for the projection-critical loads so nT
            # and the weights get the HBM bandwidth first
            maskT = persist.tile([128, NJB, N], BF16)
            maskT_r = maskT_d.rearrange("(g p) i -> p g i", p=128)
            for grp in range(4):
                d = nc.sync.dma_start(
                    out=maskT[:, ts(grp, 4), :],
                    in_=maskT_r[:, ts(grp, 4), :],
                )
                for nd in nt_dmas:
                    add_dep_helper(d.ins, nd.ins, reason="mask DMA after nT")

            # ---- PE warm-up: dummy matmuls while input DMA streams, so
            # PE_HAM unthrottles before the real projections ----
            wrm_src = persist.tile([128, 512], BF16)
            nc.vector.memset(wrm_src[:], 0.0)
            wrm_ps = psB.tile([128, 512], F32, tag="num0")
            for i in range(10):
                nc.tensor.matmul(
                    wrm_ps[:], lhsT=wrm_src[:, 0:128], rhs=wrm_src[:],
                    start=(i == 0), stop=(i == 9),
                )
            wrm_out = persist.tile([128, 512], BF16)
            nc.vector.tensor_copy(wrm_out[:], wrm_ps[:])

            # ---- projections ----
            qT = persist.tile([128, N], BF16)
            # kTz[:, h, :]: head h's dh rows at their original partitions,
            # the other head's rows zero — sim matmuls then contract over
            # all 128 partitions (a K=64 matmul leaves half the PE array
            # idle, which keeps PE_HAM throttled at 1.2 GHz globally).
            kTz = persist.tile([128, 2, N], BF16)
            nc.vector.memset(kTz[:], 0.0)
            for half in range(2):
                pps = psA.tile([128, N // 2], F32, tag=f"sim{half}")
                for isl in range(2):
                    for c in range(NC_DIM):
                        nc.tensor.matmul(
                            pps[:, ts(isl, 512)],
                            lhsT=wq[:, c, :],
                            rhs=nT[:, c, ts(half * 2 + isl, 512)],
                            start=(c == 0),
                            stop=(c == NC_DIM - 1),
                        )
                nc.scalar.activation(
                    out=qT[:, ts(half, N // 2)], in_=pps[:],
                    func=mybir.ActivationFunctionType.Identity, bias=bq[:],
                )
            for half in range(2):
                pps = psA.tile([128, N // 2], F32, tag=f"sim{half}")
                for isl in range(2):
                    for c in range(NC_DIM):
                        nc.tensor.matmul(
                            pps[:, ts(isl, 512)],
                            lhsT=wk[:, c, :],
                            rhs=nT[:, c, ts(half * 2 + isl, 512)],
                            start=(c == 0),
                            stop=(c == NC_DIM - 1),
                        )
                nc.scalar.activation(
                    out=kTz[0:64, 0, ts(half, N // 2)], in_=pps[0:64, :],
                    func=mybir.ActivationFunctionType.Identity, bias=bk[0:64, :],
                )
                nc.scalar.activation(
                    out=kTz[64:128, 1, ts(half, N // 2)], in_=pps[64:128, :],
                    func=mybir.ActivationFunctionType.Identity, bias=bk[64:128, :],
                )

            # v rows [j, dh] with a ones column appended per head (cols 0:64 =
            # head0 v, col 64 = 1, cols 65:129 = head1 v, col 129 = 1).  The
            # projection is emitted inside the attention prologue below.
            v_sb = persist.tile([128, NJB, 130], BF16)

            # ---- output projection helper (called per group below) ----
            out_r = out_d.rearrange("(g p) m -> p g m", p=128)

            def oproj_group(grp):
                osb = outp.tile([128, 4, DIM], BF16, tag="osb")
                for k in range(4):
                    ib = grp * 4 + k
                    if ib % 4 < 2:
                        ops = psA.tile([128, DIM], F32, tag=f"sim{ib % 2}")
                    else:
                        ops = psB.tile([128, DIM], F32, tag=f"num{ib % 2}")
                    nc.tensor.matmul(
                        ops[:], lhsT=attnT[:, ts(ib, 128)], rhs=wo[:],
                        start=True, stop=True,
                    )
                    if k % 2 == 0:
                        nc.vector.tensor_copy(osb[:, k, :], ops[:])
                    else:
                        nc.scalar.copy(out=osb[:, k, :], in_=ops[:])
                eng = nc.sync if grp % 2 == 0 else nc.scalar
                eng.dma_start(out=out_r[:, ts(grp, 4), :], in_=osb[:])

            # ---- attention: outer loop over i-halves, heads interleaved ----
            # sim slots are 2 banks each; per-head AV accumulators are 2
            # banks each ([65, 1024]) so all four fit in PSUM at once.  PE
            # work per (jb) unit is well under the two exp ops' ScalarE
            # time, so ScalarE streams at its floor.
            attnT = persist.tile([128, N], BF16)
            NH = N // 2
            for ihalf in range(2):
                io = ihalf * NH
                nps0 = psB.tile([65, NH], F32, tag="num0")
                nps1 = psB.tile([65, NH], F32, tag="num1")
                npss = [nps0, nps1]

                def sim_exp_mask(jb, h):
                    sps = psA.tile([128, NH], F32, tag=f"sim{h}")
                    for isl in range(2):
                        nc.tensor.matmul(
                            sps[:, ts(isl, 512)],
                            lhsT=kTz[:, h, ts(jb, 128)],
                            rhs=qT[:, bass.ds(io + isl * 512, 512)],
                            start=True,
                            stop=True,
                        )
                    pt = ptp.tile([128, NH], BF16, tag="pt")
                    nc.scalar.activation(
                        out=pt[:],
                        in_=sps[:],
                        func=mybir.ActivationFunctionType.Exp,
                        scale=SCALE,
                    )
                    nc.vector.tensor_mul(
                        pt[:], pt[:], maskT[:, jb, bass.ds(io, NH)]
                    )
                    return pt

                def av(jb, h, pt):
                    for isl in range(2):
                        nc.tensor.matmul(
                            npss[h][:, ts(isl, 512)],
                            lhsT=v_sb[:, jb, ts(h, 65)],
                            rhs=pt[:, ts(isl, 512)],
                            start=(jb == 0),
                            stop=(jb == NJB - 1),
                        )

                if ihalf == 0:
                    for jb in range(NJB):
                        vps = psA.tile([128, HG], F32, tag=f"sim{jb % 2}")
                        for c in range(NC_DIM):
                            nc.tensor.matmul(
                                vps[:],
                                lhsT=nT[:, c, ts(jb, 128)],
                                rhs=wv[:, c, :],
                                start=(c == 0),
                                stop=(c == NC_DIM - 1),
                            )
                        nc.scalar.copy(
                            out=v_sb[:, jb, 0:130].rearrange(
                                "p (h c) -> p h c", h=2
                            )[:, :, 0:64],
                            in_=vps[:].rearrange("p (h c) -> p h c", h=2),
                        )
                    nc.vector.memset(v_sb[:, :, 64:65], 1.0)
                    nc.vector.memset(v_sb[:, :, 129:130], 1.0)

                for jb in range(NJB):
                    for h in range(HEADS_PER_CORE):
                        pt = sim_exp_mask(jb, h)
                        av(jb, h, pt)
                for h in range(HEADS_PER_CORE):
                    hp = ts(h, 64)
                    if ihalf == 0:
                        # copy the AV accumulator to SBUF right away so the
                        # PSUM slot frees for the next i-half; row 64 is the
                        # softmax denominator (ones column in v_sb).
                        nsb = denp.tile([65, NH], F32, tag=f"nsb{h}")
                        nc.vector.tensor_copy(nsb[:], npss[h][:])
                        den1 = denp.tile([1, NH], F32, tag=f"den1{h}")
                        nc.sync.dma_start(out=den1[:], in_=nsb[64:65, :])
                        num_src = nsb[0:64, :]
                    else:
                        # last i-half: same SBUF staging as i-half 0 — a
                        # ScalarE den copy here would sit ahead of the
                        # early o-projection copies in the ACT FIFO and
                        # (via consolidated sem waits) stall their matmuls
                        # ~3us; DVE + the idle gpsimd DMA queue keep
                        # ScalarE free for them.
                        nsb = denp.tile([65, NH], F32, tag=f"nsb{h}")
                        nc.vector.tensor_copy(nsb[:], npss[h][:])
                        den1 = denp.tile([1, NH], F32, tag=f"den1{h}")
                        nc.gpsimd.dma_start(out=den1[:], in_=nsb[64:65, :])
                        num_src = nsb[0:64, :]
                    rec1 = denp.tile([1, NH], F32, tag=f"rec1{h}")
                    nc.vector.reciprocal_approx_fast(out=rec1[:], in_=den1[:])
                    rec = denp.tile([64, NH], F32, tag=f"rec{h}")
                    nc.gpsimd.partition_broadcast(rec[:], rec1[:])
                    nc.vector.tensor_mul(
                        attnT[hp, bass.ds(io, NH)], num_src, rec[:]
                    )


            # groups 0/1 read only i-half-0's attnT (normalized long ago)
            # — run them on sim-tag PSUM with ScalarE copies while the
            # i-half-1 norm chain (DVE/Pool) is still in flight.
            for grp in range(2):
                osb = outp.tile([128, 4, DIM], BF16, tag="osb")
                for k in range(4):
                    ib = grp * 4 + k
                    ops = psA.tile([128, DIM], F32, tag=f"sim{k % 2}")
                    nc.tensor.matmul(
                        ops[:], lhsT=attnT[:, ts(ib, 128)], rhs=wo[:],
                        start=True, stop=True,
                    )
                    nc.scalar.copy(out=osb[:, k, :], in_=ops[:])
                eng = nc.sync if grp % 2 == 0 else nc.scalar
                eng.dma_start(out=out_r[:, ts(grp, 4), :], in_=osb[:])
            for grp in range(2, 4):
                oproj_group(grp)

    # Bacc.compile runs generate_event_semaphores, which splits multi-sem
    # waits down to the 1-wait-per-instruction limit this walrus enforces.
    nc.compile()

    # Bacc's dce_regs leaves the (unread) engine-preamble register writes
    # behind at this kernel size, with deferred reg_id=-1 — walrus then
    # fails "Reg has not been allocated yet".  Nothing reads them, so any
    # valid unique per-engine id works.
    from collections import defaultdict

    next_id = defaultdict(lambda: 8)
    for a in nc.m.functions[0].allocations:
        if type(a).__name__ == "Register" and a.reg_id == -1:
            a.reg_id = next_id[str(a.engine)]
            next_id[str(a.engine)] += 1
    return nc


_NC_CACHE = None


def _get_nc():
    global _NC_CACHE
    if _NC_CACHE is None:
        _NC_CACHE = _build()
    return _NC_CACHE


def _prep_in_maps(nodes, edge_mask, wq, bq, wkv, bkv, wo, bo):
    bf16 = ml_dtypes.bfloat16
    wk_full, wv_full = wkv[:, :INNER], wkv[:, INNER:]
    bk_full, bv_full = bkv[:INNER], bkv[INNER:]
    per_batch = []
    for b in range(B):
        per_batch.append(
            (
                np.ascontiguousarray(nodes[b].T).astype(bf16),
                np.ascontiguousarray(edge_mask[b].T).astype(bf16),
            )
        )
    in_maps = []
    for core in range(NCORES):
        b, g = core // 4, core % 4
        cs = slice(g * HG, (g + 1) * HG)
        nT_b, maskT_b = per_batch[b]
        in_maps.append(
            {
                "nodesT": nT_b,
                "maskT": maskT_b,
                "wq_s": np.ascontiguousarray(wq[:, cs]).astype(bf16),
                "wk_s": np.ascontiguousarray(wk_full[:, cs]).astype(bf16),
                "wv_s": np.ascontiguousarray(wv_full[:, cs]).astype(bf16),
                "wo_s": np.ascontiguousarray(wo[cs, :]).astype(bf16),
                "bq_s": np.ascontiguousarray(bq[cs]).reshape(HG, 1).astype(np.float32),
                "bk_s": np.ascontiguousarray(bk_full[cs]).reshape(HG, 1).astype(np.float32),
            }
        )
    return in_maps


def kernel(nodes, edge_mask, wq, bq, wkv, bkv, wo, bo, _trace=False, _trace_kwargs=None):
    nodes = np.asarray(nodes, dtype=np.float32)
    edge_mask = np.asarray(edge_mask)
    wq = np.asarray(wq, dtype=np.float32)
    bq = np.asarray(bq, dtype=np.float32)
    wkv = np.asarray(wkv, dtype=np.float32)
    bkv = np.asarray(bkv, dtype=np.float32)
    wo = np.asarray(wo, dtype=np.float32)
    bo = np.asarray(bo, dtype=np.float32)

    nc = _get_nc()
    in_maps = _prep_in_maps(nodes, edge_mask, wq, bq, wkv, bkv, wo, bo)
    kw = {}
    if _trace:
        kw = dict(trace=True, **(_trace_kwargs or {}))
    res = run_bass_kernel_spmd(nc, in_maps, list(range(NCORES)), **kw)
    out = np.zeros((B, N, DIM), np.float32)
    for core in range(NCORES):
        out[core // 4] += res.results[core]["out"].astype(np.float32)
    # v-bias shifts each head's attention output by exactly bv (softmax
    # weights sum to 1), so its output contribution is the constant bv @ wo.
    bv_full = bkv[INNER:]
    out += (bv_full @ wo + bo)[None, None, :]
    if _trace:
        return out, res
    return out


# revision 46
# speedup vs baseline: 1.0215x; 1.0215x over previous
"""Sparse (masked) multi-head attention on 8 Trainium2 NeuronCores.

Problem: nodes [2,2048,512], edge_mask [2,2048,2048] (bool),
q/kv/o linear layers with H=8 heads of DH=64.

Sharding: batch x head-group.  Core c handles batch b = c//4 and head group
g = c%4 (heads 2g, 2g+1 = inner columns g*128:(g+1)*128).  Each core
computes its two heads' attention over the full sequence plus its partial
contribution to the output projection; the host sums the 4 partials per
batch and adds bo.

Per-core dataflow (all matmuls bf16 inputs, fp32 PSUM accumulation):
  qT/kT [dh=128, N]  = wq_sliceT @ nodesT (+bias)        (dh on partitions)
  v     [N, dh=128]  = nodesT.T @ wv_slice (+bias via ones-row matmul)
  per head h: simT[j,i] = kT_h.T @ qT_h                  (j on partitions)
              PT = exp(simT * DH**-0.5)   (ScalarE, free scale, bf16 out)
              PT *= maskT                  (VectorE, bf16 2x mode)
              numT[0:64,i] / den[64,i] = [v_h | 1].T @ PT  (ones col -> denom)
              attnT_h = numT * recip(den)  (recip + DMA partition-broadcast)
  out[i,:] += attnT.T @ wo_slice           (contraction over both heads)
"""
import numpy as np
import ml_dtypes

import concourse.bass as bass
import concourse.bacc as bacc
import concourse.tile as tile
from concourse import mybir
from concourse.bass_utils import run_bass_kernel_spmd
from bass_rust import add_dep_helper

B, N, DIM = 2, 2048, 512
H, DH = 8, 64
INNER = H * DH
SCALE = DH ** -0.5
NCORES = 8
HEADS_PER_CORE = 2
HG = 128            # inner columns per core (2 heads x 64)
NJB = N // 128      # 16 j-blocks
NISL = N // 512     # 4 i-slices of 512
NC_DIM = DIM // 128  # 4 contraction chunks over DIM

BF16 = mybir.dt.bfloat16
F32 = mybir.dt.float32
ts = bass.ts


def _build():
    nc = bacc.Bacc(monotonic_sem_count=0)
    nT_d = nc.declare_dram_parameter("nodesT", [DIM, N], BF16, isOutput=False)
    maskT_d = nc.declare_dram_parameter("maskT", [N, N], BF16, isOutput=False)
    wq_d = nc.declare_dram_parameter("wq_s", [DIM, HG], BF16, isOutput=False)
    wk_d = nc.declare_dram_parameter("wk_s", [DIM, HG], BF16, isOutput=False)
    wv_d = nc.declare_dram_parameter("wv_s", [DIM, HG], BF16, isOutput=False)
    wo_d = nc.declare_dram_parameter("wo_s", [HG, DIM], BF16, isOutput=False)
    bq_d = nc.declare_dram_parameter("bq_s", [HG, 1], F32, isOutput=False)
    bk_d = nc.declare_dram_parameter("bk_s", [HG, 1], F32, isOutput=False)
    out_d = nc.declare_dram_parameter("out", [N, DIM], BF16, isOutput=True)

    with tile.TileContext(nc) as tc:
        with (
            tc.tile_pool(name="persist", bufs=1) as persist,
            tc.tile_pool(name="ptp", bufs=8) as ptp,
            tc.tile_pool(name="denp", bufs=1) as denp,
            tc.tile_pool(name="outp", bufs=4) as outp,
            # PSUM is 8 banks total; two 4-bank pools, one slot each, shared
            # across phases: psA = {q/k proj, sim, even o-proj}, psB = {v proj,
            # num, odd o-proj}.
            tc.tile_pool(name="psA", bufs=1, space="PSUM") as psA,
            tc.tile_pool(name="psB", bufs=1, space="PSUM") as psB,
        ):
            # ---- loads ----
            # small weight/bias transfers issue first (they gate the
            # projections); nT chunks next; all on the Activation HWDGE
            # whose sequencer is idle until the first exp
            wq = persist.tile([128, NC_DIM, HG], BF16)
            nc.scalar.dma_start(
                out=wq[:], in_=wq_d.rearrange("(c p) m -> p c m", p=128)
            )
            nT = persist.tile([128, NC_DIM, N], BF16)
            nT_r = nT_d.rearrange("(c p) n -> p c n", p=128)
            nt_dmas = []
            for c in range(NC_DIM):
                d = nc.scalar.dma_start(out=nT[:, c, :], in_=nT_r[:, c, :])
                nt_dmas.append(d)
            wk = persist.tile([128, NC_DIM, HG], BF16)
            nc.scalar.dma_start(
                out=wk[:], in_=wk_d.rearrange("(c p) m -> p c m", p=128)
            )
            wv = persist.tile([128, NC_DIM, HG], BF16)
            nc.scalar.dma_start(
                out=wv[:], in_=wv_d.rearrange("(c p) m -> p c m", p=128)
            )
            wo = persist.tile([HG, DIM], BF16)
            nc.scalar.dma_start(out=wo[:], in_=wo_d[:])
            bq = persist.tile([HG, 1], F32)
            nc.scalar.dma_start(out=bq[:], in_=bq_d[:])
            bk = persist.tile([HG, 1], F32)
            nc.scalar.dma_start(out=bk[:], in_=bk_d[:])
            # mask transfers wait for the projection-critical loads so nT
            # and the weights get the HBM bandwidth first
            maskT = persist.tile([128, NJB, N], BF16)
            maskT_r = maskT_d.rearrange("(g p) i -> p g i", p=128)
            for grp in range(4):
                d = nc.sync.dma_start(
                    out=maskT[:, ts(grp, 4), :],
                    in_=maskT_r[:, ts(grp, 4), :],
                )
                for nd in nt_dmas:
                    add_dep_helper(d.ins, nd.ins, reason="mask DMA after nT")

            # ---- PE warm-up: dummy matmuls while input DMA streams, so
            # PE_HAM unthrottles before the real projections ----
            wrm_src = persist.tile([128, 512], BF16)
            nc.vector.memset(wrm_src[:], 0.0)
            wrm_ps = psB.tile([128, 512], F32, tag="num0")
            for i in range(10):
                nc.tensor.matmul(
                    wrm_ps[:], lhsT=wrm_src[:, 0:128], rhs=wrm_src[:],
                    start=(i == 0), stop=(i == 9),
                )
            wrm_out = persist.tile([128, 512], BF16)
            nc.vector.tensor_copy(wrm_out[:], wrm_ps[:])

            # ---- projections ----
            qT = persist.tile([128, N], BF16)
            # kTz[:, h, :]: head h's dh rows at their original partitions,
            # the other head's rows zero — sim matmuls then contract over
            # all 128 partitions (a K=64 matmul leaves half the PE array
            # idle, which keeps PE_HAM throttled at 1.2 GHz globally).
            kTz = persist.tile([128, 2, N], BF16)
            nc.vector.memset(kTz[:], 0.0)
            for half in range(2):
                pps = psA.tile([128, N // 2], F32, tag=f"sim{half}")
                for isl in range(2):
                    for c in range(NC_DIM):
                        nc.tensor.matmul(
                            pps[:, ts(isl, 512)],
                            lhsT=wq[:, c, :],
                            rhs=nT[:, c, ts(half * 2 + isl, 512)],
                            start=(c == 0),
                            stop=(c == NC_DIM - 1),
                        )
                nc.scalar.activation(
                    out=qT[:, ts(half, N // 2)], in_=pps[:],
                    func=mybir.ActivationFunctionType.Identity, bias=bq[:],
                )
            for half in range(2):
                pps = psA.tile([128, N // 2], F32, tag=f"sim{half}")
                for isl in range(2):
                    for c in range(NC_DIM):
                        nc.tensor.matmul(
                            pps[:, ts(isl, 512)],
                            lhsT=wk[:, c, :],
                            rhs=nT[:, c, ts(half * 2 + isl, 512)],
                            start=(c == 0),
                            stop=(c == NC_DIM - 1),
                        )
                nc.scalar.activation(
                    out=kTz[0:64, 0, ts(half, N // 2)], in_=pps[0:64, :],
                    func=mybir.ActivationFunctionType.Identity, bias=bk[0:64, :],
                )
                nc.scalar.activation(
                    out=kTz[64:128, 1, ts(half, N // 2)], in_=pps[64:128, :],
                    func=mybir.ActivationFunctionType.Identity, bias=bk[64:128, :],
                )

            # v rows [j, dh] with a ones column appended per head (cols 0:64 =
            # head0 v, col 64 = 1, cols 65:129 = head1 v, col 129 = 1).  The
            # projection is emitted inside the attention prologue below.
            v_sb = persist.tile([128, NJB, 130], BF16)

            # ---- output projection helper (called per group below) ----
            out_r = out_d.rearrange("(g p) m -> p g m", p=128)

            def oproj_group(grp):
                osb = outp.tile([128, 4, DIM], BF16, tag="osb")
                for k in range(4):
                    ib = grp * 4 + k
                    if ib % 4 < 2:
                        ops = psA.tile([128, DIM], F32, tag=f"sim{ib % 2}")
                    else:
                        ops = psB.tile([128, DIM], F32, tag=f"num{ib % 2}")
                    nc.tensor.matmul(
                        ops[:], lhsT=attnT[:, ts(ib, 128)], rhs=wo[:],
                        start=True, stop=True,
                    )
                    if k % 2 == 0:
                        nc.vector.tensor_copy(osb[:, k, :], ops[:])
                    else:
                        nc.scalar.copy(out=osb[:, k, :], in_=ops[:])
                eng = nc.sync if grp % 2 == 0 else nc.scalar
                eng.dma_start(out=out_r[:, ts(grp, 4), :], in_=osb[:])

            # ---- attention: outer loop over i-halves, heads interleaved ----
            # sim slots are 2 banks each; per-head AV accumulators are 2
            # banks each ([65, 1024]) so all four fit in PSUM at once.  PE
            # work per (jb) unit is well under the two exp ops' ScalarE
            # time, so ScalarE streams at its floor.
            attnT = persist.tile([128, N], BF16)
            NH = N // 2
            for ihalf in range(2):
                io = ihalf * NH
                nps0 = psB.tile([65, NH], F32, tag="num0")
                nps1 = psB.tile([65, NH], F32, tag="num1")
                npss = [nps0, nps1]

                def sim_exp_mask(jb, h):
                    sps = psA.tile([128, NH], F32, tag=f"sim{h}")
                    for isl in range(2):
                        nc.tensor.matmul(
                            sps[:, ts(isl, 512)],
                            lhsT=kTz[:, h, ts(jb, 128)],
                            rhs=qT[:, bass.ds(io + isl * 512, 512)],
                            start=True,
                            stop=True,
                        )
                    pt = ptp.tile([128, NH], BF16, tag="pt")
                    nc.scalar.activation(
                        out=pt[:],
                        in_=sps[:],
                        func=mybir.ActivationFunctionType.Exp,
                        scale=SCALE,
                    )
                    nc.vector.tensor_mul(
                        pt[:], pt[:], maskT[:, jb, bass.ds(io, NH)]
                    )
                    return pt

                def av(jb, h, pt):
                    for isl in range(2):
                        nc.tensor.matmul(
                            npss[h][:, ts(isl, 512)],
                            lhsT=v_sb[:, jb, ts(h, 65)],
                            rhs=pt[:, ts(isl, 512)],
                            start=(jb == 0),
                            stop=(jb == NJB - 1),
                        )

                if ihalf == 0:
                    for jb in range(NJB):
                        vps = psA.tile([128, HG], F32, tag=f"sim{jb % 2}")
                        for c in range(NC_DIM):
                            nc.tensor.matmul(
                                vps[:],
                                lhsT=nT[:, c, ts(jb, 128)],
                                rhs=wv[:, c, :],
                                start=(c == 0),
                                stop=(c == NC_DIM - 1),
                            )
                        nc.scalar.copy(
                            out=v_sb[:, jb, 0:130].rearrange(
                                "p (h c) -> p h c", h=2
                            )[:, :, 0:64],
                            in_=vps[:].rearrange("p (h c) -> p h c", h=2),
                        )
                    nc.vector.memset(v_sb[:, :, 64:65], 1.0)
                    nc.vector.memset(v_sb[:, :, 129:130], 1.0)

                for jb in range(NJB):
                    for h in range(HEADS_PER_CORE):
                        pt = sim_exp_mask(jb, h)
                        av(jb, h, pt)
                for h in range(HEADS_PER_CORE):
                    hp = ts(h, 64)
                    if ihalf == 0:
                        # copy the AV accumulator to SBUF right away so the
                        # PSUM slot frees for the next i-half; row 64 is the
                        # softmax denominator (ones column in v_sb).
                        nsb = denp.tile([65, NH], F32, tag=f"nsb{h}")
                        nc.vector.tensor_copy(nsb[:], npss[h][:])
                        den1 = denp.tile([1, NH], F32, tag=f"den1{h}")
                        nc.sync.dma_start(out=den1[:], in_=nsb[64:65, :])
                        num_src = nsb[0:64, :]
                    else:
                        # last i-half: nothing waits on the slot — read the
                        # accumulator directly; the denominator-row copy runs
                        # on ScalarE (idle after the last exp) so the two
                        # heads' chains overlap across engines
                        den1 = denp.tile([1, NH], F32, tag=f"den1{h}")
                        nc.scalar.copy(out=den1[:], in_=npss[h][64:65, :])
                        num_src = npss[h][0:64, :]
                    rec1 = denp.tile([1, NH], F32, tag=f"rec1{h}")
                    nc.vector.reciprocal_approx_fast(out=rec1[:], in_=den1[:])
                    rec = denp.tile([64, NH], F32, tag=f"rec{h}")
                    nc.gpsimd.partition_broadcast(rec[:], rec1[:])
                    nc.vector.tensor_mul(
                        attnT[hp, bass.ds(io, NH)], num_src, rec[:]
                    )


            # groups 0/1 read only i-half-0's attnT (normalized long ago)
            # — run them on sim-tag PSUM with ScalarE copies while the
            # i-half-1 norm chain (DVE/Pool) is still in flight.
            for grp in range(2):
                osb = outp.tile([128, 4, DIM], BF16, tag="osb")
                for k in range(4):
                    ib = grp * 4 + k
                    ops = psA.tile([128, DIM], F32, tag=f"sim{k % 2}")
                    nc.tensor.matmul(
                        ops[:], lhsT=attnT[:, ts(ib, 128)], rhs=wo[:],
                        start=True, stop=True,
                    )
                    nc.scalar.copy(out=osb[:, k, :], in_=ops[:])
                eng = nc.sync if grp % 2 == 0 else nc.scalar
                eng.dma_start(out=out_r[:, ts(grp, 4), :], in_=osb[:])
            for grp in range(2, 4):
                oproj_group(grp)

    # Bacc.compile runs generate_event_semaphores, which splits multi-sem
    # waits down to the 1-wait-per-instruction limit this walrus enforces.
    nc.compile()

    # Bacc's dce_regs leaves the (unread) engine-preamble register writes
    # behind at this kernel size, with deferred reg_id=-1 — walrus then
    # fails "Reg has not been allocated yet".  Nothing reads them, so any
    # valid unique per-engine id works.
    from collections import defaultdict

    next_id = defaultdict(lambda: 8)
    for a in nc.m.functions[0].allocations:
        if type(a).__name__ == "Register" and a.reg_id == -1:
            a.reg_id = next_id[str(a.engine)]
            next_id[str(a.engine)] += 1
    return nc


_NC_CACHE = None


def _get_nc():
    global _NC_CACHE
    if _NC_CACHE is None:
        _NC_CACHE = _build()
    return _NC_CACHE


def _prep_in_maps(nodes, edge_mask, wq, bq, wkv, bkv, wo, bo):
    bf16 = ml_dtypes.bfloat16
    wk_full, wv_full = wkv[:, :INNER], wkv[:, INNER:]
    bk_full, bv_full = bkv[:INNER], bkv[INNER:]
    per_batch = []
    for b in range(B):
        per_batch.append(
            (
                np.ascontiguousarray(nodes[b].T).astype(bf16),
                np.ascontiguousarray(edge_mask[b].T).astype(bf16),
            )
        )
    in_maps = []
    for core in range(NCORES):
        b, g = core // 4, core % 4
        cs = slice(g * HG, (g + 1) * HG)
        nT_b, maskT_b = per_batch[b]
        in_maps.append(
            {
                "nodesT": nT_b,
                "maskT": maskT_b,
                "wq_s": np.ascontiguousarray(wq[:, cs]).astype(bf16),
                "wk_s": np.ascontiguousarray(wk_full[:, cs]).astype(bf16),
                "wv_s": np.ascontiguousarray(wv_full[:, cs]).astype(bf16),
                "wo_s": np.ascontiguousarray(wo[cs, :]).astype(bf16),
                "bq_s": np.ascontiguousarray(bq[cs]).reshape(HG, 1).astype(np.float32),
                "bk_s": np.ascontiguousarray(bk_full[cs]).reshape(HG, 1).astype(np.float32),
            }
        )
    return in_maps


def kernel(nodes, edge_mask, wq, bq, wkv, bkv, wo, bo, _trace=False, _trace_kwargs=None):
    nodes = np.asarray(nodes, dtype=np.float32)
    edge_mask = np.asarray(edge_mask)
    wq = np.asarray(wq, dtype=np.float32)
    bq = np.asarray(bq, dtype=np.float32)
    wkv = np.asarray(wkv, dtype=np.float32)
    bkv = np.asarray(bkv, dtype=np.float32)
    wo = np.asarray(wo, dtype=np.float32)
    bo = np.asarray(bo, dtype=np.float32)

    nc = _get_nc()
    in_maps = _prep_in_maps(nodes, edge_mask, wq, bq, wkv, bkv, wo, bo)
    kw = {}
    if _trace:
        kw = dict(trace=True, **(_trace_kwargs or {}))
    res = run_bass_kernel_spmd(nc, in_maps, list(range(NCORES)), **kw)
    out = np.zeros((B, N, DIM), np.float32)
    for core in range(NCORES):
        out[core // 4] += res.results[core]["out"].astype(np.float32)
    # v-bias shifts each head's attention output by exactly bv (softmax
    # weights sum to 1), so its output contribution is the constant bv @ wo.
    bv_full = bkv[INNER:]
    out += (bv_full @ wo + bo)[None, None, :]
    if _trace:
        return out, res
    return out


# revision 47
# speedup vs baseline: 1.0372x; 1.0154x over previous
"""Sparse (masked) multi-head attention on 8 Trainium2 NeuronCores.

Problem: nodes [2,2048,512], edge_mask [2,2048,2048] (bool),
q/kv/o linear layers with H=8 heads of DH=64.

Sharding: batch x head-group.  Core c handles batch b = c//4 and head group
g = c%4 (heads 2g, 2g+1 = inner columns g*128:(g+1)*128).  Each core
computes its two heads' attention over the full sequence plus its partial
contribution to the output projection; the host sums the 4 partials per
batch and adds bo.

Per-core dataflow (all matmuls bf16 inputs, fp32 PSUM accumulation):
  qT/kT [dh=128, N]  = wq_sliceT @ nodesT (+bias)        (dh on partitions)
  v     [N, dh=128]  = nodesT.T @ wv_slice (+bias via ones-row matmul)
  per head h: simT[j,i] = kT_h.T @ qT_h                  (j on partitions)
              PT = exp(simT * DH**-0.5)   (ScalarE, free scale, bf16 out)
              PT *= maskT                  (VectorE, bf16 2x mode)
              numT[0:64,i] / den[64,i] = [v_h | 1].T @ PT  (ones col -> denom)
              attnT_h = numT * recip(den)  (recip + DMA partition-broadcast)
  out[i,:] += attnT.T @ wo_slice           (contraction over both heads)
"""
import numpy as np
import ml_dtypes

import concourse.bass as bass
import concourse.bacc as bacc
import concourse.tile as tile
from concourse import mybir
from concourse.bass_utils import run_bass_kernel_spmd
from bass_rust import add_dep_helper

B, N, DIM = 2, 2048, 512
H, DH = 8, 64
INNER = H * DH
SCALE = DH ** -0.5
NCORES = 8
HEADS_PER_CORE = 2
HG = 128            # inner columns per core (2 heads x 64)
NJB = N // 128      # 16 j-blocks
NISL = N // 512     # 4 i-slices of 512
NC_DIM = DIM // 128  # 4 contraction chunks over DIM

BF16 = mybir.dt.bfloat16
F32 = mybir.dt.float32
ts = bass.ts


def _build():
    nc = bacc.Bacc(monotonic_sem_count=0)
    nT_d = nc.declare_dram_parameter("nodesT", [DIM, N], BF16, isOutput=False)
    maskT_d = nc.declare_dram_parameter("maskT", [N, N], BF16, isOutput=False)
    wq_d = nc.declare_dram_parameter("wq_s", [DIM, HG], BF16, isOutput=False)
    wk_d = nc.declare_dram_parameter("wk_s", [DIM, HG], BF16, isOutput=False)
    wv_d = nc.declare_dram_parameter("wv_s", [DIM, HG], BF16, isOutput=False)
    wo_d = nc.declare_dram_parameter("wo_s", [HG, DIM], BF16, isOutput=False)
    bq_d = nc.declare_dram_parameter("bq_s", [HG, 1], F32, isOutput=False)
    bk_d = nc.declare_dram_parameter("bk_s", [HG, 1], F32, isOutput=False)
    out_d = nc.declare_dram_parameter("out", [N, DIM], BF16, isOutput=True)

    with tile.TileContext(nc) as tc:
        with (
            tc.tile_pool(name="persist", bufs=1) as persist,
            tc.tile_pool(name="ptp", bufs=12) as ptp,
            tc.tile_pool(name="denp", bufs=1) as denp,
            tc.tile_pool(name="outp", bufs=4) as outp,
            # PSUM is 8 banks total; two 4-bank pools, one slot each, shared
            # across phases: psA = {q/k proj, sim, even o-proj}, psB = {v proj,
            # num, odd o-proj}.
            tc.tile_pool(name="psA", bufs=1, space="PSUM") as psA,
            tc.tile_pool(name="psB", bufs=1, space="PSUM") as psB,
        ):
            # ---- loads ----
            # small weight/bias transfers issue first (they gate the
            # projections); nT chunks next; all on the Activation HWDGE
            # whose sequencer is idle until the first exp
            wq = persist.tile([128, NC_DIM, HG], BF16)
            nc.scalar.dma_start(
                out=wq[:], in_=wq_d.rearrange("(c p) m -> p c m", p=128)
            )
            nT = persist.tile([128, NC_DIM, N], BF16)
            nT_r = nT_d.rearrange("(c p) n -> p c n", p=128)
            nt_dmas = []
            for c in range(NC_DIM):
                d = nc.scalar.dma_start(out=nT[:, c, :], in_=nT_r[:, c, :])
                nt_dmas.append(d)
            wk = persist.tile([128, NC_DIM, HG], BF16)
            nc.scalar.dma_start(
                out=wk[:], in_=wk_d.rearrange("(c p) m -> p c m", p=128)
            )
            wv = persist.tile([128, NC_DIM, HG], BF16)
            nc.scalar.dma_start(
                out=wv[:], in_=wv_d.rearrange("(c p) m -> p c m", p=128)
            )
            wo = persist.tile([HG, DIM], BF16)
            nc.scalar.dma_start(out=wo[:], in_=wo_d[:])
            bq = persist.tile([HG, 1], F32)
            nc.scalar.dma_start(out=bq[:], in_=bq_d[:])
            bk = persist.tile([HG, 1], F32)
            nc.scalar.dma_start(out=bk[:], in_=bk_d[:])
            # mask transfers wait for the projection-critical loads so nT
            # and the weights get the HBM bandwidth first
            maskT = persist.tile([128, NJB, N], BF16)
            maskT_r = maskT_d.rearrange("(g p) i -> p g i", p=128)
            for grp in range(4):
                d = nc.sync.dma_start(
                    out=maskT[:, ts(grp, 4), :],
                    in_=maskT_r[:, ts(grp, 4), :],
                )
                for nd in nt_dmas:
                    add_dep_helper(d.ins, nd.ins, reason="mask DMA after nT")

            # ---- PE warm-up: dummy matmuls while input DMA streams, so
            # PE_HAM unthrottles before the real projections ----
            wrm_src = persist.tile([128, 512], BF16)
            nc.vector.memset(wrm_src[:], 0.0)
            wrm_ps = psB.tile([128, 512], F32, tag="num0")
            for i in range(10):
                nc.tensor.matmul(
                    wrm_ps[:], lhsT=wrm_src[:, 0:128], rhs=wrm_src[:],
                    start=(i == 0), stop=(i == 9),
                )
            wrm_out = persist.tile([128, 512], BF16)
            nc.vector.tensor_copy(wrm_out[:], wrm_ps[:])
            # dummy partition_broadcast: the Pool engine's only compute is
            # the four norm broadcasts, so the first one mid-loop pays the
            # Q7 library load (~2-7us) — pull that into the DMA-wait dead
            # time here instead.
            bc_dummy_i = persist.tile([1, 32], F32)
            nc.vector.memset(bc_dummy_i[:], 1.0)
            bc_dummy_o = persist.tile([2, 32], F32)
            nc.gpsimd.partition_broadcast(bc_dummy_o[:], bc_dummy_i[:])

            # ---- projections ----
            qT = persist.tile([128, N], BF16)
            # kTz[:, h, :]: head h's dh rows at their original partitions,
            # the other head's rows zero — sim matmuls then contract over
            # all 128 partitions (a K=64 matmul leaves half the PE array
            # idle, which keeps PE_HAM throttled at 1.2 GHz globally).
            kTz = persist.tile([128, 2, N], BF16)
            nc.vector.memset(kTz[:], 0.0)
            for half in range(2):
                pps = psA.tile([128, N // 2], F32, tag=f"sim{half}")
                for isl in range(2):
                    for c in range(NC_DIM):
                        nc.tensor.matmul(
                            pps[:, ts(isl, 512)],
                            lhsT=wq[:, c, :],
                            rhs=nT[:, c, ts(half * 2 + isl, 512)],
                            start=(c == 0),
                            stop=(c == NC_DIM - 1),
                        )
                nc.scalar.activation(
                    out=qT[:, ts(half, N // 2)], in_=pps[:],
                    func=mybir.ActivationFunctionType.Identity, bias=bq[:],
                )
            for half in range(2):
                pps = psA.tile([128, N // 2], F32, tag=f"sim{half}")
                for isl in range(2):
                    for c in range(NC_DIM):
                        nc.tensor.matmul(
                            pps[:, ts(isl, 512)],
                            lhsT=wk[:, c, :],
                            rhs=nT[:, c, ts(half * 2 + isl, 512)],
                            start=(c == 0),
                            stop=(c == NC_DIM - 1),
                        )
                nc.scalar.activation(
                    out=kTz[0:64, 0, ts(half, N // 2)], in_=pps[0:64, :],
                    func=mybir.ActivationFunctionType.Identity, bias=bk[0:64, :],
                )
                nc.scalar.activation(
                    out=kTz[64:128, 1, ts(half, N // 2)], in_=pps[64:128, :],
                    func=mybir.ActivationFunctionType.Identity, bias=bk[64:128, :],
                )

            # v rows [j, dh] with a ones column appended per head (cols 0:64 =
            # head0 v, col 64 = 1, cols 65:129 = head1 v, col 129 = 1).  The
            # projection is emitted inside the attention prologue below.
            v_sb = persist.tile([128, NJB, 130], BF16)

            # ---- output projection helper (called per group below) ----
            out_r = out_d.rearrange("(g p) m -> p g m", p=128)

            def oproj_group(grp):
                osb = outp.tile([128, 4, DIM], BF16, tag="osb")
                for k in range(4):
                    ib = grp * 4 + k
                    if ib % 4 < 2:
                        ops = psA.tile([128, DIM], F32, tag=f"sim{ib % 2}")
                    else:
                        ops = psB.tile([128, DIM], F32, tag=f"num{ib % 2}")
                    nc.tensor.matmul(
                        ops[:], lhsT=attnT[:, ts(ib, 128)], rhs=wo[:],
                        start=True, stop=True,
                    )
                    if k % 2 == 0:
                        nc.vector.tensor_copy(osb[:, k, :], ops[:])
                    else:
                        nc.scalar.copy(out=osb[:, k, :], in_=ops[:])
                eng = nc.sync if grp % 2 == 0 else nc.scalar
                eng.dma_start(out=out_r[:, ts(grp, 4), :], in_=osb[:])

            # ---- attention: outer loop over i-halves, heads interleaved ----
            # sim slots are 2 banks each; per-head AV accumulators are 2
            # banks each ([65, 1024]) so all four fit in PSUM at once.  PE
            # work per (jb) unit is well under the two exp ops' ScalarE
            # time, so ScalarE streams at its floor.
            attnT = persist.tile([128, N], BF16)
            NH = N // 2
            for ihalf in range(2):
                io = ihalf * NH
                nps0 = psB.tile([65, NH], F32, tag="num0")
                nps1 = psB.tile([65, NH], F32, tag="num1")
                npss = [nps0, nps1]

                def sim_exp_mask(jb, h):
                    sps = psA.tile([128, NH], F32, tag=f"sim{h}")
                    for isl in range(2):
                        nc.tensor.matmul(
                            sps[:, ts(isl, 512)],
                            lhsT=kTz[:, h, ts(jb, 128)],
                            rhs=qT[:, bass.ds(io + isl * 512, 512)],
                            start=True,
                            stop=True,
                        )
                    pt = ptp.tile([128, NH], BF16, tag="pt")
                    nc.scalar.activation(
                        out=pt[:],
                        in_=sps[:],
                        func=mybir.ActivationFunctionType.Exp,
                        scale=SCALE,
                    )
                    nc.vector.tensor_mul(
                        pt[:], pt[:], maskT[:, jb, bass.ds(io, NH)]
                    )
                    return pt

                def av(jb, h, pt):
                    for isl in range(2):
                        nc.tensor.matmul(
                            npss[h][:, ts(isl, 512)],
                            lhsT=v_sb[:, jb, ts(h, 65)],
                            rhs=pt[:, ts(isl, 512)],
                            start=(jb == 0),
                            stop=(jb == NJB - 1),
                        )

                if ihalf == 0:
                    for jb in range(NJB):
                        vps = psA.tile([128, HG], F32, tag=f"sim{jb % 2}")
                        for c in range(NC_DIM):
                            nc.tensor.matmul(
                                vps[:],
                                lhsT=nT[:, c, ts(jb, 128)],
                                rhs=wv[:, c, :],
                                start=(c == 0),
                                stop=(c == NC_DIM - 1),
                            )
                        nc.scalar.copy(
                            out=v_sb[:, jb, 0:130].rearrange(
                                "p (h c) -> p h c", h=2
                            )[:, :, 0:64],
                            in_=vps[:].rearrange("p (h c) -> p h c", h=2),
                        )
                    nc.vector.memset(v_sb[:, :, 64:65], 1.0)
                    nc.vector.memset(v_sb[:, :, 129:130], 1.0)

                for jb in range(NJB):
                    for h in range(HEADS_PER_CORE):
                        pt = sim_exp_mask(jb, h)
                        av(jb, h, pt)
                for h in range(HEADS_PER_CORE):
                    hp = ts(h, 64)
                    if ihalf == 0:
                        # copy the AV accumulator to SBUF right away so the
                        # PSUM slot frees for the next i-half; row 64 is the
                        # softmax denominator (ones column in v_sb).
                        nsb = denp.tile([65, NH], F32, tag=f"nsb{h}")
                        nc.vector.tensor_copy(nsb[:], npss[h][:])
                        den1 = denp.tile([1, NH], F32, tag=f"den1{h}")
                        nc.sync.dma_start(out=den1[:], in_=nsb[64:65, :])
                        num_src = nsb[0:64, :]
                    else:
                        # last i-half: nothing waits on the slot — read the
                        # accumulator directly; the denominator-row copy runs
                        # on ScalarE (idle after the last exp) so the two
                        # heads' chains overlap across engines
                        den1 = denp.tile([1, NH], F32, tag=f"den1{h}")
                        nc.scalar.copy(out=den1[:], in_=npss[h][64:65, :])
                        num_src = npss[h][0:64, :]
                    rec1 = denp.tile([1, NH], F32, tag=f"rec1{h}")
                    nc.vector.reciprocal_approx_fast(out=rec1[:], in_=den1[:])
                    rec = denp.tile([64, NH], F32, tag=f"rec{h}")
                    nc.gpsimd.partition_broadcast(rec[:], rec1[:])
                    nc.vector.tensor_mul(
                        attnT[hp, bass.ds(io, NH)], num_src, rec[:]
                    )


            # groups 0/1 read only i-half-0's attnT (normalized long ago)
            # — run them on sim-tag PSUM with ScalarE copies while the
            # i-half-1 norm chain (DVE/Pool) is still in flight.
            for grp in range(2):
                osb = outp.tile([128, 4, DIM], BF16, tag="osb")
                for k in range(4):
                    ib = grp * 4 + k
                    ops = psA.tile([128, DIM], F32, tag=f"sim{k % 2}")
                    nc.tensor.matmul(
                        ops[:], lhsT=attnT[:, ts(ib, 128)], rhs=wo[:],
                        start=True, stop=True,
                    )
                    nc.scalar.copy(out=osb[:, k, :], in_=ops[:])
                eng = nc.sync if grp % 2 == 0 else nc.scalar
                eng.dma_start(out=out_r[:, ts(grp, 4), :], in_=osb[:])
            for grp in range(2, 4):
                oproj_group(grp)

    # Bacc.compile runs generate_event_semaphores, which splits multi-sem
    # waits down to the 1-wait-per-instruction limit this walrus enforces.
    nc.compile()

    # Bacc's dce_regs leaves the (unread) engine-preamble register writes
    # behind at this kernel size, with deferred reg_id=-1 — walrus then
    # fails "Reg has not been allocated yet".  Nothing reads them, so any
    # valid unique per-engine id works.
    from collections import defaultdict

    next_id = defaultdict(lambda: 8)
    for a in nc.m.functions[0].allocations:
        if type(a).__name__ == "Register" and a.reg_id == -1:
            a.reg_id = next_id[str(a.engine)]
            next_id[str(a.engine)] += 1
    return nc


_NC_CACHE = None


def _get_nc():
    global _NC_CACHE
    if _NC_CACHE is None:
        _NC_CACHE = _build()
    return _NC_CACHE


def _prep_in_maps(nodes, edge_mask, wq, bq, wkv, bkv, wo, bo):
    bf16 = ml_dtypes.bfloat16
    wk_full, wv_full = wkv[:, :INNER], wkv[:, INNER:]
    bk_full, bv_full = bkv[:INNER], bkv[INNER:]
    per_batch = []
    for b in range(B):
        per_batch.append(
            (
                np.ascontiguousarray(nodes[b].T).astype(bf16),
                np.ascontiguousarray(edge_mask[b].T).astype(bf16),
            )
        )
    in_maps = []
    for core in range(NCORES):
        b, g = core // 4, core % 4
        cs = slice(g * HG, (g + 1) * HG)
        nT_b, maskT_b = per_batch[b]
        in_maps.append(
            {
                "nodesT": nT_b,
                "maskT": maskT_b,
                "wq_s": np.ascontiguousarray(wq[:, cs]).astype(bf16),
                "wk_s": np.ascontiguousarray(wk_full[:, cs]).astype(bf16),
                "wv_s": np.ascontiguousarray(wv_full[:, cs]).astype(bf16),
                "wo_s": np.ascontiguousarray(wo[cs, :]).astype(bf16),
                "bq_s": np.ascontiguousarray(bq[cs]).reshape(HG, 1).astype(np.float32),
                "bk_s": np.ascontiguousarray(bk_full[cs]).reshape(HG, 1).astype(np.float32),
            }
        )
    return in_maps


def kernel(nodes, edge_mask, wq, bq, wkv, bkv, wo, bo, _trace=False, _trace_kwargs=None):
    nodes = np.asarray(nodes, dtype=np.float32)
    edge_mask = np.asarray(edge_mask)
    wq = np.asarray(wq, dtype=np.float32)
    bq = np.asarray(bq, dtype=np.float32)
    wkv = np.asarray(wkv, dtype=np.float32)
    bkv = np.asarray(bkv, dtype=np.float32)
    wo = np.asarray(wo, dtype=np.float32)
    bo = np.asarray(bo, dtype=np.float32)

    nc = _get_nc()
    in_maps = _prep_in_maps(nodes, edge_mask, wq, bq, wkv, bkv, wo, bo)
    kw = {}
    if _trace:
        kw = dict(trace=True, **(_trace_kwargs or {}))
    res = run_bass_kernel_spmd(nc, in_maps, list(range(NCORES)), **kw)
    out = np.zeros((B, N, DIM), np.float32)
    for core in range(NCORES):
        out[core // 4] += res.results[core]["out"].astype(np.float32)
    # v-bias shifts each head's attention output by exactly bv (softmax
    # weights sum to 1), so its output contribution is the constant bv @ wo.
    bv_full = bkv[INNER:]
    out += (bv_full @ wo + bo)[None, None, :]
    if _trace:
        return out, res
    return out
